# revision 23
# baseline (speedup 1.0000x reference)
"""Trainium2 Bass kernel for nn_LoRALinear4bit — v14 (fp8 DoubleRow 3-pass).

out = x @ dequant_nf4(q_idx, absmax).T + (x @ A) @ B * 2.0
x [4,2048,4096] f32, q_idx [4096,4096] int32 (NF4 codes),
absmax [4096,64] f32, A [4096,16], B [16,4096].

Column/tensor parallel over 8 NeuronCores; per core 512 out-features.

Datapath: fp8e4 (e4m3) DoubleRow matmuls.  A DoubleRow matmul contracts
K=256 (two 128-k-tiles packed in the [128,2,*] sub-dim) at 0.5 cycles
per output row — 4x fp16 throughput per instruction.  e4m3's 3-bit
mantissa alone (~2.6% rms/operand) would fail the 2e-2 gate, so the
product is computed in three passes sharing one PSUM accumulation:

    y = x_hi @ W_hi  +  x_lo @ W_hi  +  x_hi @ W_lo

with x = x_hi + x_lo and W_eff = W_hi + W_lo both split hi/lo in e4m3
(second-order residuals ~0.07%; dropped x_lo@W_lo term ~0.07%).
Measured end-to-end rel err ~6e-3 vs the 2e-2 gate.

W_eff = NF4(q)*absmax*256 + 512*(A@B)^T built on device per supertile:
  q,scl,lora DMA (prefetched 2 tiles ahead, gpsimd-issued so x stripes
  don't queue-block them) -> u=(q-7.5)/7.5 (gpsimd) -> deg-8 Horner
  (first step as gpsimd ts+tt pair since walrus rejects
  TensorScalarPtr on Pool; rest DVE stt) -> *scl (DVE) -> +lora
  (gpsimd) -> W_hi=fp8 cast (ACT) -> W_lo=fp8(W-W_hi) (gpsimd).  The
  x2^8 weight scale keeps W out of the e4m3 subnormal floor; the drain
  multiplies by 2^-8.  The rank-16 lora fold 512*(A@B)^T is host prep
  (0.05% of FLOPs); supertile 0 dequants in o-halves to halve the
  head-of-pipe latency.

Scheduling: W tiles arrive ~8.7us apart while one token group consumes
all 16 in ~22us, so every group is chunked against the dequant frontier
(greedy schedule in SCHED).  Chunk closes accumulate into per-(g,m)
fp16 partials pre-scaled by 2^-8: first close on ACT (activation Copy
w/ scale), later closes and final drains on DVE via one
(psum*2^-8)+part scalar_tensor_tensor each.  No tree-sums, no PE
re-injection.  x_hi/x_lo are cast on host and streamed as fp8 stripes.
"""

import numpy as np
import ml_dtypes

B_, S_, IN, OUT = 4, 2048, 4096, 4096
TOK = B_ * S_            # 8192 tokens
NCORES = 8
OSH = OUT // NCORES      # 512 out-features per core
R = 16                   # LoRA rank
QBLOCK = 64              # bnb absmax blocksize

KT = IN // 128           # 32 k-tiles
KT2 = KT // 2            # 16 k-supertiles (K=256 each, DoubleRow)
TG = 512                 # token group
NG = TOK // TG           # 16 token groups
MPG = TG // 128          # 4 m-tiles per group
XSTR = 8                 # max k-tiles per x stripe DMA

DEG = 8                  # NF4 polynomial degree (LSQ on the 16 nodes)
HEADDEG = 6              # shorter poly for the latency-critical head tiles
WSC = 256.0              # weight scale (e4m3 subnormal avoidance)
OSC = 1.0 / WSC          # drain scale

F8NP = ml_dtypes.float8_e4m3

NF4 = np.array([
    -1.0, -0.6961928009986877, -0.5250730514526367, -0.39491748809814453,
    -0.28444138169288635, -0.18477343022823334, -0.09105003625154495, 0.0,
    0.07958029955625534, 0.16093020141124725, 0.24611230194568634,
    0.33791524171829224, 0.44070982933044434, 0.5626170039176941,
    0.6989699602127075, 1.0], dtype=np.float64)


def _poly_coeffs(deg=DEG):
    q = np.arange(16, dtype=np.float64)
    u = (q - 7.5) / 7.5
    V = np.vander(u, deg + 1, increasing=True)
    c, *_ = np.linalg.lstsq(V, NF4, rcond=None)
    return c


# (g, j0, j1) chunk units in PE order, from a greedy frontier-chaser sim
# (W[j] ready ~18+8.7j us; one group's full-K walk ~22us; 2 PSUM slots).
SCHED = [
    (0, 0, 1), (1, 0, 1), (2, 0, 1), (3, 0, 1), (4, 0, 1), (5, 0, 1),
    (6, 0, 1), (7, 0, 2), (8, 0, 2), (9, 0, 2), (10, 0, 3), (11, 0, 3),
    (12, 0, 4), (13, 0, 4), (14, 0, 5), (15, 0, 6),
    (0, 1, 7), (1, 1, 8), (2, 1, 9), (3, 1, 10), (4, 1, 11), (5, 1, 13),
    (6, 1, 15),
    (7, 2, 16), (8, 2, 16), (9, 2, 16), (10, 3, 16), (11, 3, 16),
    (12, 4, 16), (13, 4, 16), (14, 5, 16), (15, 6, 16), (0, 7, 16),
    (1, 8, 16), (2, 9, 16), (3, 10, 16), (4, 11, 16), (6, 15, 16),
    (5, 13, 16),
]

_CACHE = {}
_DBG = {}


def _build():
    key = "v30a"
    if key in _CACHE:
        return _CACHE[key]

    import concourse.bacc as bacc
    import concourse.tile as tile
    from concourse import mybir
    from concourse.bass import ts

    f32 = mybir.dt.float32
    f16 = mybir.dt.float16
    f8 = mybir.dt.float8e4
    i8 = mybir.dt.int8
    Alu = mybir.AluOpType
    DR = mybir.MatmulPerfMode.DoubleRow
    Act = mybir.ActivationFunctionType

    c = _poly_coeffs()
    ch = _poly_coeffs(HEADDEG)

    nc = bacc.Bacc("TRN2", target_bir_lowering=False, debug=False)

    xh = nc.dram_tensor("xh", [128, KT, TOK], f8, kind="ExternalInput").ap()
    xl = nc.dram_tensor("xl", [128, KT, TOK], f8, kind="ExternalInput").ap()
    qt = nc.dram_tensor("qt", [KT2, 128, 2, OSH], i8,
                        kind="ExternalInput").ap()
    scl = nc.dram_tensor("scl", [KT2, 128, 2, OSH], f16,
                         kind="ExternalInput").ap()
    lor = nc.dram_tensor("lor", [KT2, 128, 2, OSH], f16,
                         kind="ExternalInput").ap()
    out = nc.dram_tensor("out", [NG, MPG, 128, OSH], f16,
                         kind="ExternalOutput").ap()

    # gate: supertile j -> chunk units whose last tile is j
    gate = {j: [] for j in range(KT2)}
    seen = set()
    for g, j0, j1 in SCHED:
        first = g not in seen
        seen.add(g)
        gate[j1 - 1].append((g, j0, j1, first, j1 == KT2))

    with tile.TileContext(nc) as tc:
        with (
            tc.tile_pool(name="weff", bufs=1) as weff_pool,
            tc.tile_pool(name="deq", bufs=4) as deq_pool,
            tc.tile_pool(name="part", bufs=1) as part_pool,
            tc.tile_pool(name="xin", bufs=6) as x_pool,
            tc.tile_pool(name="oup", bufs=2) as o_pool,
            tc.tile_pool(name="ps", bufs=4, space="PSUM") as ps_pool,
            tc.tile_pool(name="const", bufs=1) as const_pool,
        ):
            whi = [weff_pool.tile([128, 2, OSH], f8, tag=f"whi{j}",
                                  name=f"whi{j}") for j in range(KT2)]
            wlo = [weff_pool.tile([128, 2, OSH], f8, tag=f"wlo{j}",
                                  name=f"wlo{j}") for j in range(KT2)]
            parts = {}
            deqt = {}

            # PE warmup fillers (DMA-free memset operands, start at t~0;
            # keep the clock gate warm until lora mms + first chunks).
            wz = const_pool.tile([R, 128], f16, tag="wz", name="wz")
            nc.vector.memset(wz[:], 0.0)
            wr = const_pool.tile([R, OSH], f16, tag="wr", name="wr")
            nc.vector.memset(wr[:], 0.0)
            for _ in range(40):
                wups = ps_pool.tile([128, 2, OSH], f32, tag="pp", name="pp")
                nc.tensor.matmul(wups[:, 0, :], wz[:], wr[:], start=True,
                                 stop=True)

            def emit_deq_dma(j):
                # sync-queue, prefetched ahead of the unit x stripes
                qtl = deq_pool.tile([128, 2, OSH], i8, tag="qtl", name="qtl")
                sctl = deq_pool.tile([128, 2, OSH], f16, tag="sctl",
                                     name="sctl")
                ltl = deq_pool.tile([128, 2, OSH], f16, tag="ltl",
                                    name="ltl")
                nc.sync.dma_start(out=qtl[:], in_=qt[j])
                nc.sync.dma_start(out=sctl[:], in_=scl[j])
                nc.sync.dma_start(out=ltl[:], in_=lor[j])
                deqt[j] = (qtl, sctl, ltl)

            def emit_deq(j, nsplit=1, deg=DEG, cc=None, fast=False,
                         upool=False, dvecast=False):
                cc = c if cc is None else cc
                qtl, sctl, ltl = deqt.pop(j)
                u = deq_pool.tile([128, 2, OSH], f32, tag="u", name="u")
                acc = deq_pool.tile([128, 2, OSH], f32, tag="acc", name="acc")
                wf = deq_pool.tile([128, 2, OSH], f32, tag="wf", name="wf")
                OH = OSH // nsplit
                for h in range(nsplit):
                    sli = (slice(None), slice(None),
                           slice(h * OH, (h + 1) * OH))
                    if upool:
                        # u on Pool: ACT is busy draining gate-0 closes
                        nc.gpsimd.tensor_scalar(
                            out=u[sli], in0=qtl[sli], scalar1=-7.5,
                            scalar2=1.0 / 7.5, op0=Alu.add, op1=Alu.mult)
                    else:
                        # u = (q - 7.5)/7.5 on ACT (it idles otherwise)
                        nc.scalar.activation(u[sli], qtl[sli], Act.Copy,
                                             scale=1.0 / 7.5, bias=-1.0)
                    # init + first Horner step on gpsimd (walrus rejects
                    # TensorScalarPtr on Pool, so pair ts + tt instead)
                    if upool:
                        nc.gpsimd.tensor_scalar(
                            out=u[sli], in0=qtl[sli], scalar1=-7.5,
                            scalar2=1.0 / 7.5, op0=Alu.add, op1=Alu.mult)
                    else:
                        # u = (q - 7.5)/7.5 on ACT (it idles otherwise)
                        nc.scalar.activation(u[sli], qtl[sli], Act.Copy,
                                             scale=1.0 / 7.5, bias=-1.0)
                    if fast:
                        # latency-critical head tiles: whole chain on DVE
                        # (no Pool handoffs; Pool queue congestion would
                        # delay the add/sub on the critical path)
                        nc.vector.tensor_scalar(
                            out=acc[sli], in0=u[sli], scalar1=float(cc[deg]),
                            scalar2=float(cc[deg - 1]), op0=Alu.mult,
                            op1=Alu.add)
                        nc.vector.tensor_mul(acc[sli], acc[sli], u[sli])
                    else:
                        nc.gpsimd.tensor_scalar(
                            out=acc[sli], in0=u[sli], scalar1=float(cc[deg]),
                            scalar2=float(cc[deg - 1]), op0=Alu.mult,
                            op1=Alu.add)
                        nc.gpsimd.tensor_mul(acc[sli], acc[sli], u[sli])
                    for k in range(deg - 2, 0, -1):
                        nc.vector.scalar_tensor_tensor(
                            acc[sli], acc[sli], float(cc[k]), u[sli],
                            Alu.add, Alu.mult)
                    nc.vector.scalar_tensor_tensor(
                        wf[sli], acc[sli], float(cc[0]), sctl[sli],
                        Alu.add, Alu.mult)
                    if fast:
                        nc.vector.tensor_add(wf[sli], wf[sli], ltl[sli])
                    else:
                        nc.gpsimd.tensor_add(wf[sli], wf[sli], ltl[sli])
                    if dvecast:
                        ic = nc.vector.tensor_copy(out=whi[j][sli],
                                                   in_=wf[sli])
                    else:
                        ic = nc.scalar.copy(whi[j][sli], wf[sli])
                    _DBG[ic.ins.name] = ("cast", j, h)
                    eng = nc.vector if fast else nc.gpsimd
                    isb = eng.tensor_sub(wlo[j][sli], wf[sli], whi[j][sli])
                    _DBG[isb.ins.name] = ("sub", j, h)

            def emit_unit_tail(g, j0, j1):
                # last unit: m-outer so m<3 drains/DMAs hide under the
                # next m's matmuls; only m3's drain chain ends the kernel
                xgs = []
                for s0 in range(2 * j0, 2 * j1, XSTR):
                    s1 = min(s0 + XSTR, 2 * j1)
                    xgh = x_pool.tile([128, XSTR, TG], f8, tag="xg",
                                      name="xg")
                    xgl = x_pool.tile([128, XSTR, TG], f8, tag="xg",
                                      name="xg")
                    nc.sync.dma_start(out=xgh[:, 0:s1 - s0, :],
                                      in_=xh[:, s0:s1, ts(g, TG)])
                    nc.sync.dma_start(out=xgl[:, 0:s1 - s0, :],
                                      in_=xl[:, s0:s1, ts(g, TG)])
                    xgs.append((s0, s1, xgh, xgl))
                ot = o_pool.tile([128, MPG, OSH], f16, tag="ot", name="ot")
                for mp in range(MPG // 2):
                    pp = ps_pool.tile([128, 2, OSH], f32, tag="pp",
                                      name="pp")
                    for mi in range(2):
                        m = 2 * mp + mi
                        for s0, s1, xgh, xgl in xgs:
                            for j in range(s0 // 2, s1 // 2):
                                a = 2 * j - s0
                                nc.tensor.matmul(
                                    pp[:, mi, :], xgh[:, a:a + 2, ts(m, 128)],
                                    whi[j][:], start=(j == j0), stop=False,
                                    perf_mode=DR)
                                nc.tensor.matmul(
                                    pp[:, mi, :], xgl[:, a:a + 2, ts(m, 128)],
                                    whi[j][:], start=False, stop=False,
                                    perf_mode=DR)
                                nc.tensor.matmul(
                                    pp[:, mi, :], xgh[:, a:a + 2, ts(m, 128)],
                                    wlo[j][:], start=False,
                                    stop=(j == j1 - 1), perf_mode=DR)
                    m = 2 * mp + 1
                    nc.vector.scalar_tensor_tensor(
                        ot[:, m - 1:m + 1, :], pp[:], OSC,
                        parts[(g, mp)][:], Alu.mult, Alu.add)
                    nc.scalar.dma_start(
                        out=out[g, m - 1:m + 1].transpose([1, 0, 2]),
                        in_=ot[:, m - 1:m + 1, :])

            def emit_unit(g, j0, j1, first, final):
                pps = [ps_pool.tile([128, 2, OSH], f32, tag="pp", name="pp")
                       for _ in range(MPG // 2)]
                psums = [pps[m // 2][:, m % 2, :] for m in range(MPG)]
                osp = 2 if j1 == 1 else 1
                OH = OSH // osp
                for s0 in range(2 * j0, 2 * j1, XSTR):
                    s1 = min(s0 + XSTR, 2 * j1)
                    xgh = x_pool.tile([128, XSTR, TG], f8, tag="xg",
                                      name="xg")
                    xgl = x_pool.tile([128, XSTR, TG], f8, tag="xg",
                                      name="xg")
                    nc.sync.dma_start(out=xgh[:, 0:s1 - s0, :],
                                      in_=xh[:, s0:s1, ts(g, TG)])
                    nc.sync.dma_start(out=xgl[:, 0:s1 - s0, :],
                                      in_=xl[:, s0:s1, ts(g, TG)])
                    for j in range(s0 // 2, s1 // 2):
                        a = 2 * j - s0
                        st = (j == j0)
                        sp = (j == j1 - 1)
                        for h in range(osp):
                            osl = slice(h * OH, (h + 1) * OH)
                            for m in range(MPG):
                                nc.tensor.matmul(
                                    psums[m][:, osl],
                                    xgh[:, a:a + 2, ts(m, 128)],
                                    whi[j][:, :, osl], start=st, stop=False,
                                    perf_mode=DR)
                            for m in range(MPG):
                                nc.tensor.matmul(
                                    psums[m][:, osl],
                                    xgl[:, a:a + 2, ts(m, 128)],
                                    whi[j][:, :, osl], start=False,
                                    stop=False, perf_mode=DR)
                            for m in range(MPG):
                                nc.tensor.matmul(
                                    psums[m][:, osl],
                                    xgh[:, a:a + 2, ts(m, 128)],
                                    wlo[j][:, :, osl], start=False, stop=sp,
                                    perf_mode=DR)
                if not final:
                    for mp in range(MPG // 2):
                        if first:
                            pt = part_pool.tile([128, 2, OSH], f16,
                                                tag=f"part{g}_{mp}",
                                                name=f"part{g}_{mp}")
                            parts[(g, mp)] = pt
                            # first close on ACT: part = psum-pair * 2^-8
                            nc.scalar.activation(pt[:], pps[mp][:],
                                                 Act.Copy, scale=OSC)
                        else:
                            pt = parts[(g, mp)]
                            nc.vector.scalar_tensor_tensor(
                                pt[:], pps[mp][:], OSC, pt[:],
                                Alu.mult, Alu.add)
                else:
                    ot = o_pool.tile([128, MPG, OSH], f16, tag="ot",
                                     name="ot")
                    for mp in range(MPG // 2):
                        m = 2 * mp + 1
                        if first:
                            # unsplit group: pure ACT drain, no partial
                            nc.scalar.activation(ot[:, m - 1:m + 1, :],
                                                 pps[mp][:],
                                                 Act.Copy, scale=OSC)
                        else:
                            nc.vector.scalar_tensor_tensor(
                                ot[:, m - 1:m + 1, :], pps[mp][:], OSC,
                                parts[(g, mp)][:], Alu.mult, Alu.add)
                        nc.scalar.dma_start(
                            out=out[g, m - 1:m + 1].transpose([1, 0, 2]),
                            in_=ot[:, m - 1:m + 1, :])

            for jj in range(3):
                emit_deq_dma(jj)
            for j in range(KT2):
                if j + 3 < KT2:
                    emit_deq_dma(j + 3)
                if j < 2:
                    emit_deq(j, nsplit=2, deg=HEADDEG, cc=ch, fast=True,
                             upool=(j == 1))
                else:
                    emit_deq(j)
                for g, j0, j1, first, final in gate[j]:
                    if (g, j0, j1) == SCHED[-1]:
                        emit_unit_tail(g, j0, j1)
                    else:
                        emit_unit(g, j0, j1, first, final)

    nc.compile()
    _CACHE[key] = nc
    return nc


def _prepare_in_maps(x, q_idx, absmax, lora_A, lora_B):
    x = np.asarray(x, dtype=np.float32).reshape(TOK, IN)
    q_idx = np.asarray(q_idx, dtype=np.int32)
    absmax = np.asarray(absmax, dtype=np.float32)
    lora_A = np.asarray(lora_A, dtype=np.float32)
    lora_B = np.asarray(lora_B, dtype=np.float32)

    xh8 = x.astype(F8NP)
    xl8 = (x - xh8.astype(np.float32)).astype(F8NP)
    # [128, KT, TOK]: xh[r, k, t] = x_hi[t, k*128 + r]
    xh = np.ascontiguousarray(xh8.reshape(TOK, KT, 128).transpose(2, 1, 0))
    xl = np.ascontiguousarray(xl8.reshape(TOK, KT, 128).transpose(2, 1, 0))

    qt_full = q_idx.T.astype(np.int8)                          # [IN, OUT]
    # rank-16 lora fold: 512*(A @ B) as [IN, OUT] f16
    lfull = (2.0 * WSC) * (lora_A.astype(np.float16).astype(np.float32)
                           @ lora_B.astype(np.float16).astype(np.float32))
    lfull = lfull.astype(np.float16)

    in_maps = []
    for cid in range(NCORES):
        sl = slice(cid * OSH, (cid + 1) * OSH)
        qt_c = np.ascontiguousarray(
            qt_full[:, sl].reshape(KT2, 2, 128, OSH).transpose(0, 2, 1, 3))
        scl_c = (np.repeat(np.ascontiguousarray(absmax[sl].T), QBLOCK,
                           axis=0) * WSC).astype(np.float16)   # [IN, OSH]
        scl_c = np.ascontiguousarray(
            scl_c.reshape(KT2, 2, 128, OSH).transpose(0, 2, 1, 3))
        lor_c = np.ascontiguousarray(
            lfull[:, sl].reshape(KT2, 2, 128, OSH).transpose(0, 2, 1, 3))
        in_maps.append({
            "xh": xh,
            "xl": xl,
            "qt": qt_c,
            "scl": scl_c,
            "lor": lor_c,
        })
    return in_maps


def _gather(results):
    shards = [results[cid]["out"].reshape(TOK, OSH)
              for cid in range(NCORES)]
    full = np.concatenate(shards, axis=1).astype(np.float32)   # [TOK, OUT]
    return full.reshape(B_, S_, OUT)


def kernel(x, q_idx, absmax, lora_A, lora_B):
    from concourse.bass_utils import run_bass_kernel_spmd

    nc = _build()
    in_maps = _prepare_in_maps(x, q_idx, absmax, lora_A, lora_B)
    res = run_bass_kernel_spmd(nc, in_maps, list(range(NCORES)))
    return _gather(res.results)


# revision 25
# speedup vs baseline: 1.0104x; 1.0104x over previous
"""Trainium2 Bass kernel for nn_LoRALinear4bit — v14 (fp8 DoubleRow 3-pass).

out = x @ dequant_nf4(q_idx, absmax).T + (x @ A) @ B * 2.0
x [4,2048,4096] f32, q_idx [4096,4096] int32 (NF4 codes),
absmax [4096,64] f32, A [4096,16], B [16,4096].

Column/tensor parallel over 8 NeuronCores; per core 512 out-features.

Datapath: fp8e4 (e4m3) DoubleRow matmuls.  A DoubleRow matmul contracts
K=256 (two 128-k-tiles packed in the [128,2,*] sub-dim) at 0.5 cycles
per output row — 4x fp16 throughput per instruction.  e4m3's 3-bit
mantissa alone (~2.6% rms/operand) would fail the 2e-2 gate, so the
product is computed in three passes sharing one PSUM accumulation:

    y = x_hi @ W_hi  +  x_lo @ W_hi  +  x_hi @ W_lo

with x = x_hi + x_lo and W_eff = W_hi + W_lo both split hi/lo in e4m3
(second-order residuals ~0.07%; dropped x_lo@W_lo term ~0.07%).
Measured end-to-end rel err ~6e-3 vs the 2e-2 gate.

W_eff = NF4(q)*absmax*256 + 512*(A@B)^T built on device per supertile:
  q,scl,lora DMA (prefetched 2 tiles ahead, gpsimd-issued so x stripes
  don't queue-block them) -> u=(q-7.5)/7.5 (gpsimd) -> deg-8 Horner
  (first step as gpsimd ts+tt pair since walrus rejects
  TensorScalarPtr on Pool; rest DVE stt) -> *scl (DVE) -> +lora
  (gpsimd) -> W_hi=fp8 cast (ACT) -> W_lo=fp8(W-W_hi) (gpsimd).  The
  x2^8 weight scale keeps W out of the e4m3 subnormal floor; the drain
  multiplies by 2^-8.  The rank-16 lora fold 512*(A@B)^T is host prep
  (0.05% of FLOPs); supertile 0 dequants in o-halves to halve the
  head-of-pipe latency.

Scheduling: W tiles arrive ~8.7us apart while one token group consumes
all 16 in ~22us, so every group is chunked against the dequant frontier
(greedy schedule in SCHED).  Chunk closes accumulate into per-(g,m)
fp16 partials pre-scaled by 2^-8: first close on ACT (activation Copy
w/ scale), later closes and final drains on DVE via one
(psum*2^-8)+part scalar_tensor_tensor each.  No tree-sums, no PE
re-injection.  x_hi/x_lo are cast on host and streamed as fp8 stripes.
"""

import numpy as np
import ml_dtypes

B_, S_, IN, OUT = 4, 2048, 4096, 4096
TOK = B_ * S_            # 8192 tokens
NCORES = 8
OSH = OUT // NCORES      # 512 out-features per core
R = 16                   # LoRA rank
QBLOCK = 64              # bnb absmax blocksize

KT = IN // 128           # 32 k-tiles
KT2 = KT // 2            # 16 k-supertiles (K=256 each, DoubleRow)
TG = 512                 # token group
NG = TOK // TG           # 16 token groups
MPG = TG // 128          # 4 m-tiles per group
XSTR = 8                 # max k-tiles per x stripe DMA

DEG = 8                  # NF4 polynomial degree (LSQ on the 16 nodes)
HEADDEG = 6              # shorter poly for the latency-critical head tiles
WSC = 256.0              # weight scale (e4m3 subnormal avoidance)
OSC = 1.0 / WSC          # drain scale

F8NP = ml_dtypes.float8_e4m3

NF4 = np.array([
    -1.0, -0.6961928009986877, -0.5250730514526367, -0.39491748809814453,
    -0.28444138169288635, -0.18477343022823334, -0.09105003625154495, 0.0,
    0.07958029955625534, 0.16093020141124725, 0.24611230194568634,
    0.33791524171829224, 0.44070982933044434, 0.5626170039176941,
    0.6989699602127075, 1.0], dtype=np.float64)


def _poly_coeffs(deg=DEG):
    q = np.arange(16, dtype=np.float64)
    u = (q - 7.5) / 7.5
    V = np.vander(u, deg + 1, increasing=True)
    c, *_ = np.linalg.lstsq(V, NF4, rcond=None)
    return c


# (g, j0, j1) chunk units in PE order, from a greedy frontier-chaser sim
# (W[j] ready ~18+8.7j us; one group's full-K walk ~22us; 2 PSUM slots).
SCHED = [
    (0, 0, 1), (1, 0, 1), (2, 0, 1), (3, 0, 1), (4, 0, 1), (5, 0, 1),
    (6, 0, 1), (7, 0, 2), (8, 0, 2), (9, 0, 2), (10, 0, 3), (11, 0, 3),
    (12, 0, 4), (13, 0, 4), (14, 0, 5), (15, 0, 6),
    (0, 1, 7), (1, 1, 8), (2, 1, 9), (3, 1, 10), (4, 1, 11), (5, 1, 13),
    (6, 1, 15),
    (7, 2, 16), (8, 2, 16), (9, 2, 16), (10, 3, 16), (11, 3, 16),
    (12, 4, 16), (13, 4, 16), (14, 5, 16), (15, 6, 16), (0, 7, 16),
    (1, 8, 16), (2, 9, 16), (3, 10, 16), (4, 11, 16), (6, 15, 16),
    (5, 13, 16),
]

_CACHE = {}
_DBG = {}


def _build():
    key = "v30b"
    if key in _CACHE:
        return _CACHE[key]

    import concourse.bacc as bacc
    import concourse.tile as tile
    from concourse import mybir
    from concourse.bass import ts

    f32 = mybir.dt.float32
    f16 = mybir.dt.float16
    f8 = mybir.dt.float8e4
    i8 = mybir.dt.int8
    Alu = mybir.AluOpType
    DR = mybir.MatmulPerfMode.DoubleRow
    Act = mybir.ActivationFunctionType

    c = _poly_coeffs()
    ch = _poly_coeffs(HEADDEG)

    nc = bacc.Bacc("TRN2", target_bir_lowering=False, debug=False)

    xh = nc.dram_tensor("xh", [128, KT, TOK], f8, kind="ExternalInput").ap()
    xl = nc.dram_tensor("xl", [128, KT, TOK], f8, kind="ExternalInput").ap()
    qt = nc.dram_tensor("qt", [KT2, 128, 2, OSH], i8,
                        kind="ExternalInput").ap()
    scl = nc.dram_tensor("scl", [KT2, 128, 2, OSH], f16,
                         kind="ExternalInput").ap()
    lor = nc.dram_tensor("lor", [KT2, 128, 2, OSH], f16,
                         kind="ExternalInput").ap()
    out = nc.dram_tensor("out", [NG, MPG, 128, OSH], f16,
                         kind="ExternalOutput").ap()

    # gate: supertile j -> chunk units whose last tile is j
    gate = {j: [] for j in range(KT2)}
    seen = set()
    for g, j0, j1 in SCHED:
        first = g not in seen
        seen.add(g)
        gate[j1 - 1].append((g, j0, j1, first, j1 == KT2))

    with tile.TileContext(nc) as tc:
        with (
            tc.tile_pool(name="weff", bufs=1) as weff_pool,
            tc.tile_pool(name="deq", bufs=4) as deq_pool,
            tc.tile_pool(name="part", bufs=1) as part_pool,
            tc.tile_pool(name="xin", bufs=6) as x_pool,
            tc.tile_pool(name="oup", bufs=2) as o_pool,
            tc.tile_pool(name="ps", bufs=4, space="PSUM") as ps_pool,
            tc.tile_pool(name="const", bufs=1) as const_pool,
        ):
            whi = [weff_pool.tile([128, 2, OSH], f8, tag=f"whi{j}",
                                  name=f"whi{j}") for j in range(KT2)]
            wlo = [weff_pool.tile([128, 2, OSH], f8, tag=f"wlo{j}",
                                  name=f"wlo{j}") for j in range(KT2)]
            parts = {}
            deqt = {}

            # PE warmup fillers (DMA-free memset operands, start at t~0;
            # keep the clock gate warm until lora mms + first chunks).
            wz = const_pool.tile([R, 128], f16, tag="wz", name="wz")
            nc.vector.memset(wz[:], 0.0)
            wr = const_pool.tile([R, OSH], f16, tag="wr", name="wr")
            nc.vector.memset(wr[:], 0.0)
            for _ in range(40):
                wups = ps_pool.tile([128, 2, OSH], f32, tag="pp", name="pp")
                nc.tensor.matmul(wups[:, 0, :], wz[:], wr[:], start=True,
                                 stop=True)

            def emit_deq_dma(j):
                # sync-queue, prefetched ahead of the unit x stripes
                qtl = deq_pool.tile([128, 2, OSH], i8, tag="qtl", name="qtl")
                sctl = deq_pool.tile([128, 2, OSH], f16, tag="sctl",
                                     name="sctl")
                ltl = deq_pool.tile([128, 2, OSH], f16, tag="ltl",
                                    name="ltl")
                nc.sync.dma_start(out=qtl[:], in_=qt[j])
                nc.sync.dma_start(out=sctl[:], in_=scl[j])
                nc.sync.dma_start(out=ltl[:], in_=lor[j])
                deqt[j] = (qtl, sctl, ltl)

            def emit_deq(j, nsplit=1, deg=DEG, cc=None, fast=False,
                         upool=False, dvecast=False):
                cc = c if cc is None else cc
                qtl, sctl, ltl = deqt.pop(j)
                u = deq_pool.tile([128, 2, OSH], f32, tag="u", name="u")
                acc = deq_pool.tile([128, 2, OSH], f32, tag="acc", name="acc")
                wf = deq_pool.tile([128, 2, OSH], f32, tag="wf", name="wf")
                OH = OSH // nsplit
                for h in range(nsplit):
                    sli = (slice(None), slice(None),
                           slice(h * OH, (h + 1) * OH))
                    if upool:
                        # u on Pool: ACT is busy draining gate-0 closes
                        nc.gpsimd.tensor_scalar(
                            out=u[sli], in0=qtl[sli], scalar1=-7.5,
                            scalar2=1.0 / 7.5, op0=Alu.add, op1=Alu.mult)
                    else:
                        # u = (q - 7.5)/7.5 on ACT (it idles otherwise)
                        nc.scalar.activation(u[sli], qtl[sli], Act.Copy,
                                             scale=1.0 / 7.5, bias=-1.0)
                    if fast:
                        # latency-critical head tiles: whole chain on DVE
                        # (no Pool handoffs; Pool queue congestion would
                        # delay the add/sub on the critical path)
                        nc.vector.tensor_scalar(
                            out=acc[sli], in0=u[sli], scalar1=float(cc[deg]),
                            scalar2=float(cc[deg - 1]), op0=Alu.mult,
                            op1=Alu.add)
                        nc.vector.tensor_mul(acc[sli], acc[sli], u[sli])
                    else:
                        nc.gpsimd.tensor_scalar(
                            out=acc[sli], in0=u[sli], scalar1=float(cc[deg]),
                            scalar2=float(cc[deg - 1]), op0=Alu.mult,
                            op1=Alu.add)
                        nc.gpsimd.tensor_mul(acc[sli], acc[sli], u[sli])
                    for k in range(deg - 2, 0, -1):
                        nc.vector.scalar_tensor_tensor(
                            acc[sli], acc[sli], float(cc[k]), u[sli],
                            Alu.add, Alu.mult)
                    nc.vector.scalar_tensor_tensor(
                        wf[sli], acc[sli], float(cc[0]), sctl[sli],
                        Alu.add, Alu.mult)
                    if fast:
                        nc.vector.tensor_add(wf[sli], wf[sli], ltl[sli])
                    else:
                        nc.gpsimd.tensor_add(wf[sli], wf[sli], ltl[sli])
                    if dvecast:
                        ic = nc.vector.tensor_copy(out=whi[j][sli],
                                                   in_=wf[sli])
                    else:
                        ic = nc.scalar.copy(whi[j][sli], wf[sli])
                    _DBG[ic.ins.name] = ("cast", j, h)
                    eng = nc.vector if fast else nc.gpsimd
                    isb = eng.tensor_sub(wlo[j][sli], wf[sli], whi[j][sli])
                    _DBG[isb.ins.name] = ("sub", j, h)

            def emit_unit_tail(g, j0, j1):
                # last unit: m-outer so m<3 drains/DMAs hide under the
                # next m's matmuls; only m3's drain chain ends the kernel
                xgs = []
                for s0 in range(2 * j0, 2 * j1, XSTR):
                    s1 = min(s0 + XSTR, 2 * j1)
                    xgh = x_pool.tile([128, XSTR, TG], f8, tag="xg",
                                      name="xg")
                    xgl = x_pool.tile([128, XSTR, TG], f8, tag="xg",
                                      name="xg")
                    nc.sync.dma_start(out=xgh[:, 0:s1 - s0, :],
                                      in_=xh[:, s0:s1, ts(g, TG)])
                    nc.sync.dma_start(out=xgl[:, 0:s1 - s0, :],
                                      in_=xl[:, s0:s1, ts(g, TG)])
                    xgs.append((s0, s1, xgh, xgl))
                ot = o_pool.tile([128, MPG, OSH], f16, tag="ot", name="ot")
                for mp in range(MPG // 2):
                    pp = ps_pool.tile([128, 2, OSH], f32, tag="pp",
                                      name="pp")
                    for mi in range(2):
                        m = 2 * mp + mi
                        for s0, s1, xgh, xgl in xgs:
                            for j in range(s0 // 2, s1 // 2):
                                a = 2 * j - s0
                                nc.tensor.matmul(
                                    pp[:, mi, :], xgh[:, a:a + 2, ts(m, 128)],
                                    whi[j][:], start=(j == j0), stop=False,
                                    perf_mode=DR)
                                nc.tensor.matmul(
                                    pp[:, mi, :], xgl[:, a:a + 2, ts(m, 128)],
                                    whi[j][:], start=False, stop=False,
                                    perf_mode=DR)
                                nc.tensor.matmul(
                                    pp[:, mi, :], xgh[:, a:a + 2, ts(m, 128)],
                                    wlo[j][:], start=False,
                                    stop=(j == j1 - 1), perf_mode=DR)
                    m = 2 * mp + 1
                    nc.vector.scalar_tensor_tensor(
                        ot[:, m - 1:m + 1, :], pp[:], OSC,
                        parts[(g, mp)][:], Alu.mult, Alu.add)
                    nc.scalar.dma_start(
                        out=out[g, m - 1:m + 1].transpose([1, 0, 2]),
                        in_=ot[:, m - 1:m + 1, :])

            def emit_unit(g, j0, j1, first, final):
                pps = [ps_pool.tile([128, 2, OSH], f32, tag="pp", name="pp")
                       for _ in range(MPG // 2)]
                psums = [pps[m // 2][:, m % 2, :] for m in range(MPG)]
                osp = 2 if j1 == 1 else 1
                OH = OSH // osp
                for s0 in range(2 * j0, 2 * j1, XSTR):
                    s1 = min(s0 + XSTR, 2 * j1)
                    xgh = x_pool.tile([128, XSTR, TG], f8, tag="xg",
                                      name="xg")
                    xgl = x_pool.tile([128, XSTR, TG], f8, tag="xg",
                                      name="xg")
                    nc.sync.dma_start(out=xgh[:, 0:s1 - s0, :],
                                      in_=xh[:, s0:s1, ts(g, TG)])
                    nc.sync.dma_start(out=xgl[:, 0:s1 - s0, :],
                                      in_=xl[:, s0:s1, ts(g, TG)])
                    for j in range(s0 // 2, s1 // 2):
                        a = 2 * j - s0
                        st = (j == j0)
                        sp = (j == j1 - 1)
                        for h in range(osp):
                            osl = slice(h * OH, (h + 1) * OH)
                            for m in range(MPG):
                                nc.tensor.matmul(
                                    psums[m][:, osl],
                                    xgh[:, a:a + 2, ts(m, 128)],
                                    whi[j][:, :, osl], start=st, stop=False,
                                    perf_mode=DR)
                            for m in range(MPG):
                                nc.tensor.matmul(
                                    psums[m][:, osl],
                                    xgl[:, a:a + 2, ts(m, 128)],
                                    whi[j][:, :, osl], start=False,
                                    stop=False, perf_mode=DR)
                            for m in range(MPG):
                                nc.tensor.matmul(
                                    psums[m][:, osl],
                                    xgh[:, a:a + 2, ts(m, 128)],
                                    wlo[j][:, :, osl], start=False, stop=sp,
                                    perf_mode=DR)
                if not final:
                    for mp in range(MPG // 2):
                        if first:
                            pt = part_pool.tile([128, 2, OSH], f16,
                                                tag=f"part{g}_{mp}",
                                                name=f"part{g}_{mp}")
                            parts[(g, mp)] = pt
                            # first close on ACT: part = psum-pair * 2^-8
                            nc.scalar.activation(pt[:], pps[mp][:],
                                                 Act.Copy, scale=OSC)
                        else:
                            pt = parts[(g, mp)]
                            nc.vector.scalar_tensor_tensor(
                                pt[:], pps[mp][:], OSC, pt[:],
                                Alu.mult, Alu.add)
                else:
                    ot = o_pool.tile([128, MPG, OSH], f16, tag="ot",
                                     name="ot")
                    for mp in range(MPG // 2):
                        m = 2 * mp + 1
                        if first:
                            # unsplit group: pure ACT drain, no partial
                            nc.scalar.activation(ot[:, m - 1:m + 1, :],
                                                 pps[mp][:],
                                                 Act.Copy, scale=OSC)
                        else:
                            nc.vector.scalar_tensor_tensor(
                                ot[:, m - 1:m + 1, :], pps[mp][:], OSC,
                                parts[(g, mp)][:], Alu.mult, Alu.add)
                        nc.scalar.dma_start(
                            out=out[g, m - 1:m + 1].transpose([1, 0, 2]),
                            in_=ot[:, m - 1:m + 1, :])

            for jj in range(3):
                emit_deq_dma(jj)
            for j in range(KT2):
                if j + 3 < KT2:
                    emit_deq_dma(j + 3)
                if j < 2:
                    emit_deq(j, nsplit=2, deg=HEADDEG, cc=ch, fast=True,
                             upool=(j == 1))
                else:
                    emit_deq(j)
                for g, j0, j1, first, final in gate[j]:
                    if (g, j0, j1) == SCHED[-1]:
                        emit_unit_tail(g, j0, j1)
                    else:
                        emit_unit(g, j0, j1, first, final)

    nc.compile()
    _CACHE[key] = nc
    return nc


def _prepare_in_maps(x, q_idx, absmax, lora_A, lora_B):
    x = np.asarray(x, dtype=np.float32).reshape(TOK, IN)
    q_idx = np.asarray(q_idx, dtype=np.int32)
    absmax = np.asarray(absmax, dtype=np.float32)
    lora_A = np.asarray(lora_A, dtype=np.float32)
    lora_B = np.asarray(lora_B, dtype=np.float32)

    xh8 = x.astype(F8NP)
    xl8 = (x - xh8.astype(np.float32)).astype(F8NP)
    # [128, KT, TOK]: xh[r, k, t] = x_hi[t, k*128 + r]
    xh = np.ascontiguousarray(xh8.reshape(TOK, KT, 128).transpose(2, 1, 0))
    xl = np.ascontiguousarray(xl8.reshape(TOK, KT, 128).transpose(2, 1, 0))

    qt_full = q_idx.T.astype(np.int8)                          # [IN, OUT]
    # rank-16 lora fold: 512*(A @ B) as [IN, OUT] f16
    lfull = (2.0 * WSC) * (lora_A.astype(np.float16).astype(np.float32)
                           @ lora_B.astype(np.float16).astype(np.float32))
    lfull = lfull.astype(np.float16)

    in_maps = []
    for cid in range(NCORES):
        sl = slice(cid * OSH, (cid + 1) * OSH)
        qt_c = np.ascontiguousarray(
            qt_full[:, sl].reshape(KT2, 2, 128, OSH).transpose(0, 2, 1, 3))
        scl_c = (np.repeat(np.ascontiguousarray(absmax[sl].T), QBLOCK,
                           axis=0) * WSC).astype(np.float16)   # [IN, OSH]
        scl_c = np.ascontiguousarray(
            scl_c.reshape(KT2, 2, 128, OSH).transpose(0, 2, 1, 3))
        lor_c = np.ascontiguousarray(
            lfull[:, sl].reshape(KT2, 2, 128, OSH).transpose(0, 2, 1, 3))
        in_maps.append({
            "xh": xh,
            "xl": xl,
            "qt": qt_c,
            "scl": scl_c,
            "lor": lor_c,
        })
    return in_maps


def _gather(results):
    shards = [results[cid]["out"].reshape(TOK, OSH)
              for cid in range(NCORES)]
    full = np.concatenate(shards, axis=1).astype(np.float32)   # [TOK, OUT]
    return full.reshape(B_, S_, OUT)


def kernel(x, q_idx, absmax, lora_A, lora_B):
    from concourse.bass_utils import run_bass_kernel_spmd

    nc = _build()
    in_maps = _prepare_in_maps(x, q_idx, absmax, lora_A, lora_B)
    res = run_bass_kernel_spmd(nc, in_maps, list(range(NCORES)))
    return _gather(res.results)
